# revision 28
# baseline (speedup 1.0000x reference)
"""Trainium2 Bass kernel for bilinear forward-warp splatting (scatter_memory).

Per batch element b (data-parallel over 8 NeuronCores):
    wy = y0 + dt*fy;  wx = x0 + dt*fx          (dt = tref - i)
    out[y, x] = sum_p v_p * tent(wy_p - y) * tent(wx_p - x)
for channels v in {1, fy, fx}, tent(u) = max(0, 1-|u|), then
wf = splat(w*f) / (splat(w) + eps).

Structure (v3, fully on-chip): bands of BH=16 rows, column-interleave IL=8
(chunks of 128 points = 16 rows x 8 cols).  Per band, fused DVE passes build
the y-tent matrices (lhsT, uniform window mYu = BH+2*dymax+2) and the x-tent
rhs channels; the TensorEngine accumulates sum_p tentY^T (x) [tX, tX*fy,
tX*fx] into 5 PSUM segments of exactly 128 grid cols each (chunk windows are
split at segment boundaries - column splits don't change stream cost).  ACT
copies segments into a flat band slab [mYu, W*3].  Adjacent band windows
overlap in y; a per-band SBUF->SBUF accumulate-DMA cascades the overlap down
(slab_b += slab_{b-1} shifted 16 rows), after which the top 16 rows of each
slab are final.  Strips are DMA-gathered into 128-row groups, normalized
(reciprocal_approx_fast), and written straight to the outputs.  No DRAM
scratch, no cross-DMA DRAM hazards; every dependency is SBUF-tracked.
"""

import os
import sys
import math

import numpy as np

for _p in ("/opt/trn_rl_repo", "/root/.axon_site/_ro/trn_rl_repo"):
    if os.path.isdir(_p) and _p not in sys.path:
        sys.path.insert(0, _p)

from contextlib import ExitStack

import concourse.bass as bass
import concourse.bacc as bacc
import concourse.tile as tile
from concourse import mybir
from concourse.ap import AP
from concourse.bass_utils import run_bass_kernel_spmd

H, W = 480, 640
NCORES = 8
F32 = mybir.dt.float32
BF16 = mybir.dt.bfloat16
Alu = mybir.AluOpType
Act = mybir.ActivationFunctionType

BH = 16              # band height
IL = 8               # column interleave (chunk = BH x IL = 128 points)
NPAIR = W // IL      # column groups (80)
NBAND = H // BH      # 30
NBLK = (H + 127) // 128
SP = 16              # column groups per segment (128 grid cols)
NSEG = NPAIR // SP   # 5
SEGW = IL * SP       # 128 grid cols per segment
EPS = 1e-9
BIG = 4.0e6

_OPS = None


def _ops():
    """Register (once) the custom DVE ops: TENT, YPUSH, XPUSH."""
    global _OPS
    if _OPS is not None:
        return _OPS
    from concourse import dve_ops as dvo
    from concourse.dve_spec import Spec, Src0, Src1, Zero, One, C0, C1, maxx, relu, lower
    from concourse.dve_uop import DveOpSpec

    def reg(name, spec, rd1):
        for op in dvo.OPS:
            if op.name == name:
                return op
        row = dvo._CUSTOM_DVE_ROW_BASE + len(dvo.OPS)
        shas = {}
        for ver in ("v3", "v4"):
            shas[ver] = DveOpSpec(name=name, opcode=row, uops=lower(spec, ver=ver),
                                  rd1_en=rd1).sha(ver)
        op = dvo.DveOp(name, spec, subdim=False, uops_sha=shas)
        dvo.OPS.append(op)
        dvo._SUB_OPCODE_FOR_NAME[name] = row
        dvo.CUSTOM_DVE_SPECS[name] = spec
        return op

    tent = reg("TENT_ANT", Spec(
        body=relu(One - maxx(Src0 - Src1, Src1 - Src0)),
        reference=lambda in0, in1, s0, s1, imm2: np.maximum(
            0.0, 1.0 - np.abs(in0 - in1)),
    ), True)
    # out = in0 + s1*((in0 < 0) + (in0 > s0))
    ypush = reg("YPUSH_ANT", Spec(
        body=Src0 + C1 * ((Src0 < Zero) + (Src0 > C0)),
        reference=lambda in0, in1, s0, s1, imm2: in0 + s1 * (
            (in0 < 0).astype(np.float32) + (in0 > s0).astype(np.float32)),
    ), False)
    # out = in1 + s1*((in0 < 0) + (in0 > s0))
    xpush = reg("XPUSH_ANT", Spec(
        body=Src1 + C1 * ((Src0 < Zero) + (Src0 > C0)),
        reference=lambda in0, in1, s0, s1, imm2: in1 + s1 * (
            (in0 < 0).astype(np.float32) + (in0 > s0).astype(np.float32)),
    ), True)
    _OPS = (tent, ypush, xpush)
    return _OPS


def _v(ap, dims, extra_off=0, parts=None):
    """Manual AP view: keep ap's partition pair, replace free dims."""
    ppair = [ap.ap[0][0], ap.ap[0][1] if parts is None else parts]
    return AP(tensor=ap.tensor, offset=ap.offset + extra_off,
              ap=[ppair] + [list(d) for d in dims])


def _vsrc(ap, rows):
    """Source view [rows, IL, NPAIR] of a [.., W] tile: elem (i, j) = col IL*j+i."""
    return _v(ap[:rows], [[1, IL], [IL, NPAIR]])


def _build_program(dt, dys, dxss, H=H, W=W):
    """dys: per-band y half-windows; dxss: per-(band, segment) x half-windows.
    y uses the global max (uniform windows keep the band cascade aligned)."""
    TENT, YPUSH, XPUSH = _ops()
    dymax = max(dys)
    dxmax = max(max(r) for r in dxss)
    mYu = BH + 2 * dymax + 2
    assert mYu <= 64, f"dymax {dymax} too large"
    assert 2 * dxmax + 2 + IL < SEGW
    OY = dymax + 1                                     # win0_b = BH*b - OY
    YWMAX = (mYu + 1) // 2 * 2                         # even
    XWMAX = (2 * dxmax + 2 + IL + 1) // 2 * 2          # even
    W3 = 3 * W

    nc = bacc.Bacc("TRN2", target_bir_lowering=False, debug=False)
    fy_in = nc.declare_dram_parameter("fy", [H, W], F32, isOutput=False)
    fx_in = nc.declare_dram_parameter("fx", [H, W], F32, isOutput=False)
    o_wfx = nc.declare_dram_parameter("out_wfx", [H, W], F32, isOutput=True)
    o_wfy = nc.declare_dram_parameter("out_wfy", [H, W], F32, isOutput=True)

    # strip bookkeeping (host): per band, finalized grid rows and group splits
    def strip_pieces(b):
        """[(slab_row0, grid_row0, nrows)] for band b's finalized strip."""
        r0, r1 = BH * b - OY, BH * b + BH - OY
        if b == NBAND - 1:
            r1 = BH * b - OY + mYu                     # tail: rest of last slab
        lo = max(r0, 0)
        hi = min(r1, H)
        out = []
        r = lo
        while r < hi:
            ln = min(hi - r, 128 - r % 128)            # split at group bounds
            out.append((r - r0, r, ln))
            r += ln
        return out

    ngroups = (H + 127) // 128
    group_last_band = [0] * ngroups
    for b in range(NBAND):
        for _, gr, ln in strip_pieces(b):
            for g in range(gr // 128, (gr + ln - 1) // 128 + 1):
                group_last_band[g] = max(group_last_band[g], b)

    with ExitStack() as ctx:
        tc = ctx.enter_context(tile.TileContext(nc))
        singles = ctx.enter_context(tc.tile_pool(name="singles", bufs=1))

        # ---- constant ramps (f32, exact integers) ----
        NY = H + 2 * (dymax + 2) + 4
        NX = W + 2 * (dxmax + 2) + 4
        PADY = dymax + 2
        ioY = singles.tile([128, NY], F32)   # value = idx - PADY
        ioX = singles.tile([128, NX], F32)   # value = idx - (dxmax + 1)
        y0f = singles.tile([128, NBLK], F32)
        nc.gpsimd.iota(ioY[:], pattern=[[1, NY]], base=-PADY, channel_multiplier=0,
                       allow_small_or_imprecise_dtypes=True)
        nc.gpsimd.iota(ioX[:], pattern=[[1, NX]], base=-(dxmax + 1), channel_multiplier=0,
                       allow_small_or_imprecise_dtypes=True)
        nc.gpsimd.iota(y0f[:], pattern=[[128, NBLK]], base=0, channel_multiplier=1,
                       allow_small_or_imprecise_dtypes=True)
        x0v = ioX[:, dxmax + 1:dxmax + 1 + W]  # values 0..W-1

        # zero operands for PSUM-clearing matmuls
        z_l = singles.tile([16, 128], BF16)
        z_r = singles.tile([16, 512], BF16)
        nc.gpsimd.memset(z_l[:], 0.0)
        nc.gpsimd.memset(z_r[:], 0.0)

        # ---- prep (emitted per 128-row block, interleaved with the bands
        # that consume it so the PE starts as soon as block 0 is ready) ----
        # PS layout [128, plane(4), NBLK, IL, NPAIR]: planes wyM, wx, fy, fx
        PS = singles.tile([128, 4, NBLK, IL, NPAIR], F32)

        mains = ExitStack()
        inpool = mains.enter_context(tc.tile_pool(name="inpool", bufs=2))
        preptmp = mains.enter_context(tc.tile_pool(name="preptmp", bufs=1))
        bandp = mains.enter_context(tc.tile_pool(name="bandp", bufs=4))
        tentp = mains.enter_context(tc.tile_pool(name="tentp", bufs=3))
        build = mains.enter_context(tc.tile_pool(name="build", bufs=2))
        slabp = mains.enter_context(tc.tile_pool(name="slabp", bufs=4))
        outp = mains.enter_context(tc.tile_pool(name="outp", bufs=2))
        finp = mains.enter_context(tc.tile_pool(name="finp", bufs=1))
        psump = mains.enter_context(tc.tile_pool(name="psump", bufs=8, space="PSUM"))

        def prep_block(blk):
            rows = min(128, H - 128 * blk)
            in_fy = inpool.tile([128, W], F32, tag="in_fy")
            in_fx = inpool.tile([128, W], F32, tag="in_fx")
            nc.sync.dma_start(out=in_fy[:rows], in_=fy_in.ap()[128 * blk:128 * blk + rows])
            nc.sync.dma_start(out=in_fx[:rows], in_=fx_in.ap()[128 * blk:128 * blk + rows])
            wy = preptmp.tile([128, W], F32, tag="wy")
            wx = preptmp.tile([128, W], F32, tag="wx")

            def pview(pl):  # packed-dest parity view [rows, IL, NPAIR]
                return _v(PS[:rows, pl, blk], [[NPAIR, IL], [1, NPAIR]])

            nc.vector.tensor_scalar(out=wy[:rows], in0=in_fy[:rows], scalar1=dt,
                                    scalar2=y0f[:rows, blk:blk + 1], op0=Alu.mult, op1=Alu.add)
            nc.vector._custom_dve(YPUSH, out=wy[:rows], in0=wy[:rows],
                                  s0=float(H - 1), s1=BIG)
            nc.vector.scalar_tensor_tensor(out=wx[:rows], in0=in_fx[:rows], scalar=dt,
                                           in1=x0v[:rows], op0=Alu.mult, op1=Alu.add)
            # wyM = wy + BIG*(wx out of range), written straight into PS
            nc.vector._custom_dve(XPUSH, out=pview(0), in0=_vsrc(wx, rows),
                                  in1=_vsrc(wy, rows), s0=float(W - 1), s1=BIG)
            nc.scalar.activation(out=pview(1), in_=_vsrc(wx, rows), func=Act.Copy)
            nc.scalar.activation(out=pview(2), in_=_vsrc(in_fy, rows), func=Act.Copy)
            nc.scalar.activation(out=pview(3), in_=_vsrc(in_fx, rows), func=Act.Copy)

        slabs = [None] * NBAND
        outbufs = [None] * ngroups

        def group_buf(g):
            if outbufs[g] is None:
                ob = outp.tile([128, W, 3], F32, tag="outbuf")
                outbufs[g] = ob
            return outbufs[g]

        def finalize_group(g):
            ob = outbufs[g]
            rows = min(128, H - 128 * g)
            rec = finp.tile([128, W], F32, tag="rec")
            ofy = finp.tile([128, W], F32, tag="ofy")
            ofx = finp.tile([128, W], F32, tag="ofx")
            nc.vector.tensor_scalar(out=rec[:rows], in0=ob[:rows, :, 0],
                                    scalar1=EPS, scalar2=None, op0=Alu.add)
            nc.vector.reciprocal_approx_fast(out=rec[:rows], in_=rec[:rows])
            nc.vector.tensor_tensor(out=ofy[:rows], in0=ob[:rows, :, 1],
                                    in1=rec[:rows], op=Alu.mult)
            nc.vector.tensor_tensor(out=ofx[:rows], in0=ob[:rows, :, 2],
                                    in1=rec[:rows], op=Alu.mult)
            nc.sync.dma_start(out=o_wfx.ap()[128 * g:128 * g + rows], in_=ofx[:rows])
            nc.sync.dma_start(out=o_wfy.ap()[128 * g:128 * g + rows], in_=ofy[:rows])

        prep_block(0)
        for b in range(NBAND):
            a = BH * b
            blk, p0 = divmod(a, 128)
            if p0 == 64 and blk + 1 < NBLK:
                prep_block(blk + 1)
            dxs_b = dxss[b]
            dx = max(dxs_b)
            XW = 2 * dx + 2 + IL

            bandC = bandp.tile([128, 4, NPAIR], F32, tag="bandC")
            for i in range(IL):
                nc.sync.dma_start(out=bandC[BH * i:BH * (i + 1)],
                                  in_=PS[p0:p0 + BH, :, blk, i])

            tentY = tentp.tile([128, NPAIR, YWMAX], BF16, tag="tentY")
            rhs = build.tile([128, NPAIR, 3, XWMAX], BF16, tag="rhs")

            # y tents over the uniform window [a-OY, a-OY+mYu)
            nc.vector._custom_dve(
                TENT,
                out=_v(tentY[:], [[YWMAX, NPAIR], [1, mYu]]),
                in0=_v(ioY[:, PADY + a - OY:], [[0, NPAIR], [1, mYu]]),
                in1=_v(bandC[:, 0], [[1, NPAIR], [0, mYu]]))
            # expand fy/fx into ch1/ch2 (ACT, full band width)
            nc.scalar.activation(out=_v(rhs[:], [[3 * XWMAX, NPAIR], [1, XW]], extra_off=XWMAX),
                                 in_=_v(bandC[:, 2], [[1, NPAIR], [0, XW]]),
                                 func=Act.Copy)
            nc.scalar.activation(out=_v(rhs[:], [[3 * XWMAX, NPAIR], [1, XW]], extra_off=2 * XWMAX),
                                 in_=_v(bandC[:, 3], [[1, NPAIR], [0, XW]]),
                                 func=Act.Copy)
            # x tents into rhs channel 0 + channel muls, per segment (regional dx)
            for t in range(NSEG):
                dxt = dxs_b[t]
                XWt = 2 * dxt + 2 + IL
                off_t = 3 * XWMAX * SP * t
                nc.vector._custom_dve(
                    TENT,
                    out=_v(rhs[:], [[3 * XWMAX, SP], [1, XWt]], extra_off=off_t),
                    in0=_v(ioX[:, dxmax - dxt + SEGW * t:], [[IL, SP], [1, XWt]]),
                    in1=_v(bandC[:, 1], [[1, SP], [0, XWt]], extra_off=SP * t))
                for ch in (1, 2):
                    nc.vector.tensor_tensor(
                        out=_v(rhs[:], [[3 * XWMAX, SP], [1, XWt]], extra_off=off_t + ch * XWMAX),
                        in0=_v(rhs[:], [[3 * XWMAX, SP], [1, XWt]], extra_off=off_t + ch * XWMAX),
                        in1=_v(rhs[:], [[3 * XWMAX, SP], [1, XWt]], extra_off=off_t), op=Alu.mult)

            # 5 segments of exactly 128 grid cols; chunk windows split at
            # segment boundaries (and clipped at the image edge)
            psegs = []
            for s in range(NSEG):
                ptile = psump.tile([128, 512], F32, tag="pseg")
                psegs.append((ptile, 0))
                nc.tensor.matmul(ptile[:mYu, :3 * SEGW], lhsT=z_l[:, :mYu],
                                 rhs=z_r[:, :3 * SEGW], start=True, stop=False)
            nmm = [1] * NSEG      # zero-mm counted; track last matmul per seg
            total_mm = [0] * NSEG
            for jj in range(NPAIR):
                dxj = dxs_b[jj // SP]
                wlo = max(0, IL * jj - dxj - 1)
                whi = min(W, IL * jj + IL + dxj + 1)
                s0 = wlo // SEGW
                s1 = (whi - 1) // SEGW
                for s in range(s0, s1 + 1):
                    total_mm[s] += 1
            for jj in range(NPAIR):
                dxj = dxs_b[jj // SP]
                win_l = IL * jj - dxj - 1             # window start (may be <0)
                wlo = max(0, win_l)
                whi = min(W, IL * jj + IL + dxj + 1)
                s0 = wlo // SEGW
                s1 = (whi - 1) // SEGW
                for s in range(s0, s1 + 1):
                    c_lo = max(wlo, SEGW * s)
                    c_hi = min(whi, SEGW * (s + 1))
                    ptile, off = psegs[s]
                    rhs_j = _v(rhs[:], [[1, c_hi - c_lo], [XWMAX, 3]],
                               extra_off=jj * 3 * XWMAX + (c_lo - win_l))
                    nmm[s] += 1
                    nc.tensor.matmul(
                        ptile[off + 0:off + mYu,
                              3 * (c_lo - SEGW * s):3 * (c_hi - SEGW * s)],
                        lhsT=_v(tentY[:], [[1, mYu]], extra_off=jj * YWMAX),
                        rhs=rhs_j,
                        start=False, stop=(nmm[s] == total_mm[s] + 1))

            slab = slabp.tile([128, W, 3], F32, tag="slab")
            slabs[b] = slab
            for s in range(NSEG):
                ptile, off = psegs[s]
                nc.scalar.activation(
                    out=_v(slab[:mYu], [[1, 3 * SEGW]], extra_off=3 * SEGW * s),
                    in_=ptile[off:off + mYu, :3 * SEGW], func=Act.Copy)

            # cascade: slab += previous band's slab shifted down 16 rows
            if b > 0:
                nc.gpsimd.dma_start(out=slab[:mYu - BH],
                                    in_=slabs[b - 1][BH:mYu],
                                    accum_op=Alu.add)
                slabs[b - 1] = None
            # finalized strip(s) -> output group buffers
            for sr, gr, ln in strip_pieces(b):
                g = gr // 128
                ob = group_buf(g)
                nc.scalar.dma_start(out=ob[gr - 128 * g:gr - 128 * g + ln],
                                    in_=slab[sr:sr + ln])
            for g in range(ngroups):
                if group_last_band[g] == b and outbufs[g] is not None:
                    finalize_group(g)

        mains.close()

    nc.compile()
    return nc


_PROG_CACHE = {}


def _get_program(dt, dys, dxss):
    key = (float(dt), tuple(dys), tuple(map(tuple, dxss)))
    if key not in _PROG_CACHE:
        _PROG_CACHE[key] = _build_program(dt, dys, dxss)
    return _PROG_CACHE[key]


def _window_params(fy, fx, dt):
    """Per-band displacement bounds (max over batch)."""
    ady = np.abs(dt) * np.abs(fy).max(axis=0)      # [H, W]
    adx = np.abs(dt) * np.abs(fx).max(axis=0)
    dys, dxss = [], []
    for b in range(NBAND):
        sl = slice(BH * b, BH * (b + 1))
        dys.append(max(2, int(math.ceil(float(ady[sl].max())))))
        dxss.append(tuple(
            max(2, int(math.ceil(float(adx[sl, SEGW * t:SEGW * (t + 1)].max()))))
            for t in range(NSEG)))
    return dys, dxss


def kernel(flow_maps_x, flow_maps_y, i=0, tref=4):
    i = int(i)
    tref = int(tref)
    dt = float(tref - i)
    B = flow_maps_x.shape[0]
    assert B <= NCORES, f"batch {B} > {NCORES} cores not supported"
    fx = np.ascontiguousarray(flow_maps_x[:, i]).astype(np.float32)
    fy = np.ascontiguousarray(flow_maps_y[:, i]).astype(np.float32)

    dys, dxs = _window_params(fy, fx, dt)
    nc = _get_program(dt, dys, dxs)
    in_maps = [{"fy": fy[b], "fx": fx[b]} for b in range(B)]
    res = run_bass_kernel_spmd(nc, in_maps, list(range(B)))
    wfx = np.stack([res.results[b]["out_wfx"] for b in range(B)])[:, None]
    wfy = np.stack([res.results[b]["out_wfy"] for b in range(B)])[:, None]
    return wfx.astype(np.float32), wfy.astype(np.float32)


def _ensure_ntff_hook():
    """The agent image lacks antenv.axon_hooks; synthesize it from trn_agent_boot."""
    import types
    try:
        import antenv.axon_hooks  # noqa: F401
        return
    except ImportError:
        pass
    from trn_agent_boot.trn_boot import _ntff_profile_via_ctypes
    hook = _ntff_profile_via_ctypes("/opt/axon/libaxon_pjrt.so")
    m = types.ModuleType("antenv.axon_hooks")
    m.get_axon_ntff_profile_hook = lambda: hook
    m.set_axon_ntff_profile_hook = lambda h: None
    sys.modules["antenv.axon_hooks"] = m


def timed_run(np_inputs):
    """Run once with NTFF tracing; return HW exec time in ns."""
    _ensure_ntff_hook()
    i = int(np_inputs["i"]); tref = int(np_inputs["tref"])
    dt = float(tref - i)
    fx = np.ascontiguousarray(np_inputs["flow_maps_x"][:, i]).astype(np.float32)
    fy = np.ascontiguousarray(np_inputs["flow_maps_y"][:, i]).astype(np.float32)
    B = fx.shape[0]
    dys, dxs = _window_params(fy, fx, dt)
    nc = _get_program(dt, dys, dxs)
    in_maps = [{"fy": fy[b], "fx": fx[b]} for b in range(B)]
    res = run_bass_kernel_spmd(nc, in_maps, list(range(B)), trace=True)
    return res.exec_time_ns


if __name__ == "__main__":
    rng = np.random.default_rng(0)
    fmx = rng.standard_normal((8, 4, H, W), dtype=np.float32)
    fmy = rng.standard_normal((8, 4, H, W), dtype=np.float32)
    ox, oy = kernel(fmx, fmy, 0, 4)
    print(ox.shape, oy.shape, ox.dtype)


# revision 29
# speedup vs baseline: 1.0012x; 1.0012x over previous
"""Trainium2 Bass kernel for bilinear forward-warp splatting (scatter_memory).

Per batch element b (data-parallel over 8 NeuronCores):
    wy = y0 + dt*fy;  wx = x0 + dt*fx          (dt = tref - i)
    out[y, x] = sum_p v_p * tent(wy_p - y) * tent(wx_p - x)
for channels v in {1, fy, fx}, tent(u) = max(0, 1-|u|), then
wf = splat(w*f) / (splat(w) + eps).

Structure (v3, fully on-chip): bands of BH=16 rows, column-interleave IL=8
(chunks of 128 points = 16 rows x 8 cols).  Per band, fused DVE passes build
the y-tent matrices (lhsT, uniform window mYu = BH+2*dymax+2) and the x-tent
rhs channels; the TensorEngine accumulates sum_p tentY^T (x) [tX, tX*fy,
tX*fx] into 5 PSUM segments of exactly 128 grid cols each (chunk windows are
split at segment boundaries - column splits don't change stream cost).  ACT
copies segments into a flat band slab [mYu, W*3].  Adjacent band windows
overlap in y; a per-band SBUF->SBUF accumulate-DMA cascades the overlap down
(slab_b += slab_{b-1} shifted 16 rows), after which the top 16 rows of each
slab are final.  Strips are DMA-gathered into 128-row groups, normalized
(reciprocal_approx_fast), and written straight to the outputs.  No DRAM
scratch, no cross-DMA DRAM hazards; every dependency is SBUF-tracked.
"""

import os
import sys
import math

import numpy as np

for _p in ("/opt/trn_rl_repo", "/root/.axon_site/_ro/trn_rl_repo"):
    if os.path.isdir(_p) and _p not in sys.path:
        sys.path.insert(0, _p)

from contextlib import ExitStack

import concourse.bass as bass
import concourse.bacc as bacc
import concourse.tile as tile
from concourse import mybir
from concourse.ap import AP
from concourse.bass_utils import run_bass_kernel_spmd

H, W = 480, 640
NCORES = 8
F32 = mybir.dt.float32
BF16 = mybir.dt.bfloat16
Alu = mybir.AluOpType
Act = mybir.ActivationFunctionType

BH = 16              # band height
IL = 8               # column interleave (chunk = BH x IL = 128 points)
NPAIR = W // IL      # column groups (80)
NBAND = H // BH      # 30
NBLK = (H + 127) // 128
SP = 16              # column groups per segment (128 grid cols)
NSEG = NPAIR // SP   # 5
SEGW = IL * SP       # 128 grid cols per segment
EPS = 1e-9
BIG = 4.0e6

_OPS = None


def _ops():
    """Register (once) the custom DVE ops: TENT, YPUSH, XPUSH."""
    global _OPS
    if _OPS is not None:
        return _OPS
    from concourse import dve_ops as dvo
    from concourse.dve_spec import Spec, Src0, Src1, Zero, One, C0, C1, maxx, relu, lower
    from concourse.dve_uop import DveOpSpec

    def reg(name, spec, rd1):
        for op in dvo.OPS:
            if op.name == name:
                return op
        row = dvo._CUSTOM_DVE_ROW_BASE + len(dvo.OPS)
        shas = {}
        for ver in ("v3", "v4"):
            shas[ver] = DveOpSpec(name=name, opcode=row, uops=lower(spec, ver=ver),
                                  rd1_en=rd1).sha(ver)
        op = dvo.DveOp(name, spec, subdim=False, uops_sha=shas)
        dvo.OPS.append(op)
        dvo._SUB_OPCODE_FOR_NAME[name] = row
        dvo.CUSTOM_DVE_SPECS[name] = spec
        return op

    tent = reg("TENT_ANT", Spec(
        body=relu(One - maxx(Src0 - Src1, Src1 - Src0)),
        reference=lambda in0, in1, s0, s1, imm2: np.maximum(
            0.0, 1.0 - np.abs(in0 - in1)),
    ), True)
    # out = in0 + s1*((in0 < 0) + (in0 > s0))
    ypush = reg("YPUSH_ANT", Spec(
        body=Src0 + C1 * ((Src0 < Zero) + (Src0 > C0)),
        reference=lambda in0, in1, s0, s1, imm2: in0 + s1 * (
            (in0 < 0).astype(np.float32) + (in0 > s0).astype(np.float32)),
    ), False)
    # out = in1 + s1*((in0 < 0) + (in0 > s0))
    xpush = reg("XPUSH_ANT", Spec(
        body=Src1 + C1 * ((Src0 < Zero) + (Src0 > C0)),
        reference=lambda in0, in1, s0, s1, imm2: in1 + s1 * (
            (in0 < 0).astype(np.float32) + (in0 > s0).astype(np.float32)),
    ), True)
    _OPS = (tent, ypush, xpush)
    return _OPS


def _v(ap, dims, extra_off=0, parts=None):
    """Manual AP view: keep ap's partition pair, replace free dims."""
    ppair = [ap.ap[0][0], ap.ap[0][1] if parts is None else parts]
    return AP(tensor=ap.tensor, offset=ap.offset + extra_off,
              ap=[ppair] + [list(d) for d in dims])


def _vsrc(ap, rows):
    """Source view [rows, IL, NPAIR] of a [.., W] tile: elem (i, j) = col IL*j+i."""
    return _v(ap[:rows], [[1, IL], [IL, NPAIR]])


def _build_program(dt, dys, dxss, H=H, W=W):
    """dys: per-band y half-windows; dxss: per-(band, segment) x half-windows.
    y uses the global max (uniform windows keep the band cascade aligned)."""
    TENT, YPUSH, XPUSH = _ops()
    dymax = max(dys)
    dxmax = max(max(r) for r in dxss)
    mYu = BH + 2 * dymax + 2
    assert mYu <= 64, f"dymax {dymax} too large"
    assert 2 * dxmax + 2 + IL < SEGW
    OY = dymax + 1                                     # win0_b = BH*b - OY
    YWMAX = (mYu + 1) // 2 * 2                         # even
    XWMAX = (2 * dxmax + 2 + IL + 1) // 2 * 2          # even
    W3 = 3 * W

    nc = bacc.Bacc("TRN2", target_bir_lowering=False, debug=False)
    fy_in = nc.declare_dram_parameter("fy", [H, W], F32, isOutput=False)
    fx_in = nc.declare_dram_parameter("fx", [H, W], F32, isOutput=False)
    o_wfx = nc.declare_dram_parameter("out_wfx", [H, W], F32, isOutput=True)
    o_wfy = nc.declare_dram_parameter("out_wfy", [H, W], F32, isOutput=True)

    # strip bookkeeping (host): per band, finalized grid rows and group splits
    def strip_pieces(b):
        """[(slab_row0, grid_row0, nrows)] for band b's finalized strip."""
        r0, r1 = BH * b - OY, BH * b + BH - OY
        if b == NBAND - 1:
            r1 = BH * b - OY + mYu                     # tail: rest of last slab
        lo = max(r0, 0)
        hi = min(r1, H)
        out = []
        r = lo
        while r < hi:
            ln = min(hi - r, 128 - r % 128)            # split at group bounds
            out.append((r - r0, r, ln))
            r += ln
        return out

    ngroups = (H + 127) // 128
    group_last_band = [0] * ngroups
    for b in range(NBAND):
        for _, gr, ln in strip_pieces(b):
            for g in range(gr // 128, (gr + ln - 1) // 128 + 1):
                group_last_band[g] = max(group_last_band[g], b)

    with ExitStack() as ctx:
        tc = ctx.enter_context(tile.TileContext(nc))
        singles = ctx.enter_context(tc.tile_pool(name="singles", bufs=1))

        # ---- constant ramps (f32, exact integers) ----
        NY = H + 2 * (dymax + 2) + 4
        NX = W + 2 * (dxmax + 2) + 4
        PADY = dymax + 2
        ioY = singles.tile([128, NY], F32)   # value = idx - PADY
        ioX = singles.tile([128, NX], F32)   # value = idx - (dxmax + 1)
        y0f = singles.tile([128, NBLK], F32)
        nc.gpsimd.iota(ioY[:], pattern=[[1, NY]], base=-PADY, channel_multiplier=0,
                       allow_small_or_imprecise_dtypes=True)
        nc.gpsimd.iota(ioX[:], pattern=[[1, NX]], base=-(dxmax + 1), channel_multiplier=0,
                       allow_small_or_imprecise_dtypes=True)
        nc.gpsimd.iota(y0f[:], pattern=[[128, NBLK]], base=0, channel_multiplier=1,
                       allow_small_or_imprecise_dtypes=True)
        x0v = ioX[:, dxmax + 1:dxmax + 1 + W]  # values 0..W-1

        # zero operands for PSUM-clearing matmuls
        z_l = singles.tile([16, 128], BF16)
        z_r = singles.tile([16, 512], BF16)
        nc.gpsimd.memset(z_l[:], 0.0)
        nc.gpsimd.memset(z_r[:], 0.0)

        # ---- prep (emitted per 128-row block, interleaved with the bands
        # that consume it so the PE starts as soon as block 0 is ready) ----
        # PS layout [128, plane(4), NBLK, IL, NPAIR]: planes wyM, wx, fy, fx
        PS = singles.tile([128, 4, NBLK, IL, NPAIR], F32)

        mains = ExitStack()
        inpool = mains.enter_context(tc.tile_pool(name="inpool", bufs=2))
        preptmp = mains.enter_context(tc.tile_pool(name="preptmp", bufs=1))
        bandp = mains.enter_context(tc.tile_pool(name="bandp", bufs=3))
        tentp = mains.enter_context(tc.tile_pool(name="tentp", bufs=3))
        build = mains.enter_context(tc.tile_pool(name="build", bufs=2))
        slabp = mains.enter_context(tc.tile_pool(name="slabp", bufs=3))
        outp = mains.enter_context(tc.tile_pool(name="outp", bufs=2))
        finp = mains.enter_context(tc.tile_pool(name="finp", bufs=1))
        psump = mains.enter_context(tc.tile_pool(name="psump", bufs=8, space="PSUM"))

        def prep_block(blk):
            rows = min(128, H - 128 * blk)
            in_fy = inpool.tile([128, W], F32, tag="in_fy")
            in_fx = inpool.tile([128, W], F32, tag="in_fx")
            nc.sync.dma_start(out=in_fy[:rows], in_=fy_in.ap()[128 * blk:128 * blk + rows])
            nc.sync.dma_start(out=in_fx[:rows], in_=fx_in.ap()[128 * blk:128 * blk + rows])
            wy = preptmp.tile([128, W], F32, tag="wy")
            wx = preptmp.tile([128, W], F32, tag="wx")

            def pview(pl):  # packed-dest parity view [rows, IL, NPAIR]
                return _v(PS[:rows, pl, blk], [[NPAIR, IL], [1, NPAIR]])

            nc.vector.tensor_scalar(out=wy[:rows], in0=in_fy[:rows], scalar1=dt,
                                    scalar2=y0f[:rows, blk:blk + 1], op0=Alu.mult, op1=Alu.add)
            nc.vector._custom_dve(YPUSH, out=wy[:rows], in0=wy[:rows],
                                  s0=float(H - 1), s1=BIG)
            nc.vector.scalar_tensor_tensor(out=wx[:rows], in0=in_fx[:rows], scalar=dt,
                                           in1=x0v[:rows], op0=Alu.mult, op1=Alu.add)
            # wyM = wy + BIG*(wx out of range), written straight into PS
            nc.vector._custom_dve(XPUSH, out=pview(0), in0=_vsrc(wx, rows),
                                  in1=_vsrc(wy, rows), s0=float(W - 1), s1=BIG)
            nc.scalar.activation(out=pview(1), in_=_vsrc(wx, rows), func=Act.Copy)
            nc.scalar.activation(out=pview(2), in_=_vsrc(in_fy, rows), func=Act.Copy)
            nc.scalar.activation(out=pview(3), in_=_vsrc(in_fx, rows), func=Act.Copy)

        slabs = [None] * NBAND
        outbufs = [None] * ngroups

        def group_buf(g):
            if outbufs[g] is None:
                ob = outp.tile([128, W, 3], F32, tag="outbuf")
                outbufs[g] = ob
            return outbufs[g]

        def finalize_group(g):
            ob = outbufs[g]
            rows = min(128, H - 128 * g)
            rec = finp.tile([128, W], F32, tag="rec")
            ofy = finp.tile([128, W], F32, tag="ofy")
            ofx = finp.tile([128, W], F32, tag="ofx")
            nc.vector.tensor_scalar(out=rec[:rows], in0=ob[:rows, :, 0],
                                    scalar1=EPS, scalar2=None, op0=Alu.add)
            nc.vector.reciprocal_approx_fast(out=rec[:rows], in_=rec[:rows])
            nc.vector.tensor_tensor(out=ofy[:rows], in0=ob[:rows, :, 1],
                                    in1=rec[:rows], op=Alu.mult)
            nc.vector.tensor_tensor(out=ofx[:rows], in0=ob[:rows, :, 2],
                                    in1=rec[:rows], op=Alu.mult)
            nc.sync.dma_start(out=o_wfx.ap()[128 * g:128 * g + rows], in_=ofx[:rows])
            nc.sync.dma_start(out=o_wfy.ap()[128 * g:128 * g + rows], in_=ofy[:rows])

        prep_block(0)
        for b in range(NBAND):
            a = BH * b
            blk, p0 = divmod(a, 128)
            if p0 == 64 and blk + 1 < NBLK:
                prep_block(blk + 1)
            dxs_b = dxss[b]
            dx = max(dxs_b)
            XW = 2 * dx + 2 + IL

            bandC = bandp.tile([128, 4, NPAIR], F32, tag="bandC")
            for i in range(IL):
                nc.sync.dma_start(out=bandC[BH * i:BH * (i + 1)],
                                  in_=PS[p0:p0 + BH, :, blk, i])

            tentY = tentp.tile([128, NPAIR, YWMAX], BF16, tag="tentY")
            rhs = build.tile([128, NPAIR, 3, XWMAX], BF16, tag="rhs")

            # y tents over the uniform window [a-OY, a-OY+mYu)
            nc.vector._custom_dve(
                TENT,
                out=_v(tentY[:], [[YWMAX, NPAIR], [1, mYu]]),
                in0=_v(ioY[:, PADY + a - OY:], [[0, NPAIR], [1, mYu]]),
                in1=_v(bandC[:, 0], [[1, NPAIR], [0, mYu]]))
            # expand fy/fx into ch1/ch2 (ACT, full band width)
            nc.scalar.activation(out=_v(rhs[:], [[3 * XWMAX, NPAIR], [1, XW]], extra_off=XWMAX),
                                 in_=_v(bandC[:, 2], [[1, NPAIR], [0, XW]]),
                                 func=Act.Copy)
            nc.scalar.activation(out=_v(rhs[:], [[3 * XWMAX, NPAIR], [1, XW]], extra_off=2 * XWMAX),
                                 in_=_v(bandC[:, 3], [[1, NPAIR], [0, XW]]),
                                 func=Act.Copy)
            # x tents into rhs channel 0 + channel muls, per segment (regional dx)
            for t in range(NSEG):
                dxt = dxs_b[t]
                XWt = 2 * dxt + 2 + IL
                off_t = 3 * XWMAX * SP * t
                nc.vector._custom_dve(
                    TENT,
                    out=_v(rhs[:], [[3 * XWMAX, SP], [1, XWt]], extra_off=off_t),
                    in0=_v(ioX[:, dxmax - dxt + SEGW * t:], [[IL, SP], [1, XWt]]),
                    in1=_v(bandC[:, 1], [[1, SP], [0, XWt]], extra_off=SP * t))
                for ch in (1, 2):
                    nc.vector.tensor_tensor(
                        out=_v(rhs[:], [[3 * XWMAX, SP], [1, XWt]], extra_off=off_t + ch * XWMAX),
                        in0=_v(rhs[:], [[3 * XWMAX, SP], [1, XWt]], extra_off=off_t + ch * XWMAX),
                        in1=_v(rhs[:], [[3 * XWMAX, SP], [1, XWt]], extra_off=off_t), op=Alu.mult)

            # 5 segments of exactly 128 grid cols; chunk windows split at
            # segment boundaries (and clipped at the image edge)
            psegs = []
            for s in range(NSEG):
                ptile = psump.tile([128, 512], F32, tag="pseg")
                psegs.append((ptile, 0))
                nc.tensor.matmul(ptile[:mYu, :3 * SEGW], lhsT=z_l[:, :mYu],
                                 rhs=z_r[:, :3 * SEGW], start=True, stop=False)
            nmm = [1] * NSEG      # zero-mm counted; track last matmul per seg
            total_mm = [0] * NSEG
            for jj in range(NPAIR):
                dxj = dxs_b[jj // SP]
                wlo = max(0, IL * jj - dxj - 1)
                whi = min(W, IL * jj + IL + dxj + 1)
                s0 = wlo // SEGW
                s1 = (whi - 1) // SEGW
                for s in range(s0, s1 + 1):
                    total_mm[s] += 1
            for jj in range(NPAIR):
                dxj = dxs_b[jj // SP]
                win_l = IL * jj - dxj - 1             # window start (may be <0)
                wlo = max(0, win_l)
                whi = min(W, IL * jj + IL + dxj + 1)
                s0 = wlo // SEGW
                s1 = (whi - 1) // SEGW
                for s in range(s0, s1 + 1):
                    c_lo = max(wlo, SEGW * s)
                    c_hi = min(whi, SEGW * (s + 1))
                    ptile, off = psegs[s]
                    rhs_j = _v(rhs[:], [[1, c_hi - c_lo], [XWMAX, 3]],
                               extra_off=jj * 3 * XWMAX + (c_lo - win_l))
                    nmm[s] += 1
                    nc.tensor.matmul(
                        ptile[off + 0:off + mYu,
                              3 * (c_lo - SEGW * s):3 * (c_hi - SEGW * s)],
                        lhsT=_v(tentY[:], [[1, mYu]], extra_off=jj * YWMAX),
                        rhs=rhs_j,
                        start=False, stop=(nmm[s] == total_mm[s] + 1))

            slab = slabp.tile([128, W, 3], F32, tag="slab")
            slabs[b] = slab
            for s in range(NSEG):
                ptile, off = psegs[s]
                nc.scalar.activation(
                    out=_v(slab[:mYu], [[1, 3 * SEGW]], extra_off=3 * SEGW * s),
                    in_=ptile[off:off + mYu, :3 * SEGW], func=Act.Copy)

            # cascade: slab += previous band's slab shifted down 16 rows
            if b > 0:
                nc.gpsimd.dma_start(out=slab[:mYu - BH],
                                    in_=slabs[b - 1][BH:mYu],
                                    accum_op=Alu.add)
                slabs[b - 1] = None
            # finalized strip(s) -> output group buffers
            for sr, gr, ln in strip_pieces(b):
                g = gr // 128
                ob = group_buf(g)
                nc.scalar.dma_start(out=ob[gr - 128 * g:gr - 128 * g + ln],
                                    in_=slab[sr:sr + ln])
            for g in range(ngroups):
                if group_last_band[g] == b and outbufs[g] is not None:
                    finalize_group(g)

        mains.close()

    nc.compile()
    return nc


_PROG_CACHE = {}


def _get_program(dt, dys, dxss):
    key = (float(dt), tuple(dys), tuple(map(tuple, dxss)))
    if key not in _PROG_CACHE:
        _PROG_CACHE[key] = _build_program(dt, dys, dxss)
    return _PROG_CACHE[key]


def _window_params(fy, fx, dt):
    """Per-band displacement bounds (max over batch)."""
    ady = np.abs(dt) * np.abs(fy).max(axis=0)      # [H, W]
    adx = np.abs(dt) * np.abs(fx).max(axis=0)
    dys, dxss = [], []
    for b in range(NBAND):
        sl = slice(BH * b, BH * (b + 1))
        dys.append(max(2, int(math.ceil(float(ady[sl].max())))))
        dxss.append(tuple(
            max(2, int(math.ceil(float(adx[sl, SEGW * t:SEGW * (t + 1)].max()))))
            for t in range(NSEG)))
    return dys, dxss


def kernel(flow_maps_x, flow_maps_y, i=0, tref=4):
    i = int(i)
    tref = int(tref)
    dt = float(tref - i)
    B = flow_maps_x.shape[0]
    assert B <= NCORES, f"batch {B} > {NCORES} cores not supported"
    fx = np.ascontiguousarray(flow_maps_x[:, i]).astype(np.float32)
    fy = np.ascontiguousarray(flow_maps_y[:, i]).astype(np.float32)

    dys, dxs = _window_params(fy, fx, dt)
    nc = _get_program(dt, dys, dxs)
    in_maps = [{"fy": fy[b], "fx": fx[b]} for b in range(B)]
    res = run_bass_kernel_spmd(nc, in_maps, list(range(B)))
    wfx = np.stack([res.results[b]["out_wfx"] for b in range(B)])[:, None]
    wfy = np.stack([res.results[b]["out_wfy"] for b in range(B)])[:, None]
    return wfx.astype(np.float32), wfy.astype(np.float32)


def _ensure_ntff_hook():
    """The agent image lacks antenv.axon_hooks; synthesize it from trn_agent_boot."""
    import types
    try:
        import antenv.axon_hooks  # noqa: F401
        return
    except ImportError:
        pass
    from trn_agent_boot.trn_boot import _ntff_profile_via_ctypes
    hook = _ntff_profile_via_ctypes("/opt/axon/libaxon_pjrt.so")
    m = types.ModuleType("antenv.axon_hooks")
    m.get_axon_ntff_profile_hook = lambda: hook
    m.set_axon_ntff_profile_hook = lambda h: None
    sys.modules["antenv.axon_hooks"] = m


def timed_run(np_inputs):
    """Run once with NTFF tracing; return HW exec time in ns."""
    _ensure_ntff_hook()
    i = int(np_inputs["i"]); tref = int(np_inputs["tref"])
    dt = float(tref - i)
    fx = np.ascontiguousarray(np_inputs["flow_maps_x"][:, i]).astype(np.float32)
    fy = np.ascontiguousarray(np_inputs["flow_maps_y"][:, i]).astype(np.float32)
    B = fx.shape[0]
    dys, dxs = _window_params(fy, fx, dt)
    nc = _get_program(dt, dys, dxs)
    in_maps = [{"fy": fy[b], "fx": fx[b]} for b in range(B)]
    res = run_bass_kernel_spmd(nc, in_maps, list(range(B)), trace=True)
    return res.exec_time_ns


if __name__ == "__main__":
    rng = np.random.default_rng(0)
    fmx = rng.standard_normal((8, 4, H, W), dtype=np.float32)
    fmy = rng.standard_normal((8, 4, H, W), dtype=np.float32)
    ox, oy = kernel(fmx, fmy, 0, 4)
    print(ox.shape, oy.shape, ox.dtype)
